# revision 1
# baseline (speedup 1.0000x reference)
"""LSTM autoencoder (4-layer + TimeDistributed Dense) on 8 TRN2 NeuronCores.

Sharding: data-parallel over batch (B=256 -> 32 samples/core), weights
replicated. Per-core layout keeps everything "transposed": states are
[H partitions, batch free], so the recurrent matmul is
  z^T[gate] = W[:, gate]^T @ h^T   (weights stationary, state moving, N=32)
and the gate nonlinearities/cell updates run on [H, 32] tiles.

Per 16-step block, the input-side (Wk) matmuls for all 16 steps are batched
into one N=512 matmul per gate plane (PSUM bank), exploiting has_written
accumulation: the recurrent matmuls then accumulate per-step N=32 slices
on top.

Biases are folded in via a ones-row augmentation of the moving operand
(K -> K+1) on whichever side has K < 128.

Gate plane order in PSUM is (g, i, f, o) — g first so one sigmoid
activation covers planes 1:4. relu(c) == c identically because c >= 0 by
induction (g >= 0 post-relu, i,f in (0,1), c0 = 0), so h = o * c.

All input DMAs (weights packed into one tensor, x in one shot) happen
up-front on a minimal number of DMA queues: per-block DMAs would rotate
across all 8 DMA-queue semaphores and push per-matmul sync-wait counts
past the ISA limit (matmul instructions carry at most 2 waits).
"""

import numpy as np

B, T, F, H1, H2 = 256, 512, 64, 128, 64
NCORES = 8
BC = B // NCORES          # 32 samples per core
NT = T * BC               # 16384 columns in time-major (t, b) layout
SB = 16                   # recurrence steps per PSUM block
NBLK = T // SB            # 32 blocks
BLKC = SB * BC            # 512 columns per block
PERM = [2, 0, 1, 3]       # keras (i,f,g,o) -> planes (g,i,f,o)

# wpack column offsets: (name, rows, cols)
WSEGS = [("wk1", 65, 512), ("wr1", 128, 512), ("wk2", 128, 256),
         ("wr2", 65, 256), ("wd1k", 65, 256), ("wd1r", 64, 256),
         ("wd2k", 65, 512), ("wd2r", 128, 512), ("wout", 128, 64),
         ("bout", 64, 1)]
WOFF = {}
_o = 0
for _n, _p, _c in WSEGS:
    WOFF[_n] = _o
    _o += _c
WCOLS = _o

_CACHE = {}


def _build():
    import concourse.bass as bass
    import concourse.mybir as mybir
    import concourse.tile as tile

    f32 = mybir.dt.float32
    AF = mybir.ActivationFunctionType

    nc = bass.Bass()

    xa = nc.dram_tensor("xa", [F + 1, NT], f32, kind="ExternalInput")
    wp_d = nc.dram_tensor("wpack", [128, WCOLS], f32, kind="ExternalInput")
    out_d = nc.dram_tensor("out", [F, NT], f32, kind="ExternalOutput")

    with tile.TileContext(nc) as tc:
        with (
            tc.tile_pool(name="singles", bufs=1) as singles,
            tc.tile_pool(name="work", bufs=4) as work,
            tc.tile_pool(name="psum", bufs=2, space="PSUM") as psum_pool,
        ):
            wp = singles.tile([128, WCOLS], f32, tag="wp")
            nc.sync.dma_start(wp[:], wp_d[:])

            def wslice(name, rows, g, H):
                o = WOFF[name]
                return wp[0:rows, o + g * H: o + (g + 1) * H]

            # --- state buffers ---
            # big_a serves as h1_seq (phases A,B) then h4_seq (phases D,E).
            # Column layout: col (t+1)*32 .. +32 holds h_t; cols 0:32 zero.
            # xh3 serves as the full x input (phase A, cols t*32 directly)
            # then as h3_seq (phases C,D, cols shifted by +BC).
            big_a = singles.tile([H1, NT + BC], f32, tag="big_a")
            xh3 = singles.tile([H2 + 1, NT + BC], f32, tag="xh3")
            h2a = singles.tile([H2 + 1, BC], f32, tag="h2a")
            z_rep = singles.tile([H2 + 1, BLKC], f32, tag="z_rep")
            c_big = singles.tile([H1, BC], f32, tag="c_big")
            c_sm = singles.tile([H2, BC], f32, tag="c_sm")

            nc.sync.dma_start(xh3[:, 0:NT], xa[:])

            def lstm_step(nc, ps, cs, H, wr_g, hprev, c_t, h_out):
                """Emit one recurrence step given psum block ps / col slice cs."""
                for g in range(4):
                    nc.tensor.matmul(
                        ps[:, g, cs], wr_g(g), hprev,
                        start=False, stop=True, skip_group_check=True,
                    )
                act = work.tile([H, 3, BC], f32, tag="act")
                nc.scalar.activation(act[:], ps[:, 1:4, cs], AF.Sigmoid)
                rg = work.tile([H, BC], f32, tag="rg")
                nc.scalar.activation(rg[:], ps[:, 0, cs], AF.Relu)
                u = work.tile([H, BC], f32, tag="u")
                nc.vector.tensor_mul(u[:], rg[:], act[:, 0, :])
                nc.vector.tensor_mul(c_t[:], act[:, 1, :], c_t[:])
                nc.vector.tensor_add(c_t[:], c_t[:], u[:])
                nc.vector.tensor_mul(h_out, act[:, 2, :], c_t[:])

            def lstm_phase(H, wk_name, wk_rows, wr_name, wr_rows,
                           x_of_blk, hseq, c_t):
                """One LSTM layer over all T steps.

                x_of_blk(blk) -> rhs AP [wk_rows, 512] for the batched
                input-side matmul. hseq: [H(+aug), NT+BC] sequence buffer;
                col (t+1)*BC holds h_t (rows 0:H written by the step).
                """
                nc.vector.memset(c_t[:], 0.0)
                nc.vector.memset(hseq[0:H, 0:BC], 0.0)
                wr_g = lambda g: wslice(wr_name, wr_rows, g, H)
                for blk in range(NBLK):
                    ps = psum_pool.tile([H, 4, BLKC], f32, tag="ps")
                    xr = x_of_blk(blk)
                    for g in range(4):
                        nc.tensor.matmul(
                            ps[:, g, :], wslice(wk_name, wk_rows, g, H), xr,
                            start=True, stop=False, skip_group_check=True,
                        )
                    for s in range(SB):
                        t = blk * SB + s
                        cs = slice(s * BC, (s + 1) * BC)
                        lstm_step(
                            nc, ps, cs, H, wr_g,
                            hseq[0:H, t * BC:(t + 1) * BC], c_t,
                            hseq[0:H, (t + 1) * BC:(t + 2) * BC],
                        )

            # --- phase A: encoder L1 (x -> h1_seq in big_a) ---
            lstm_phase(
                H1, "wk1", 65, "wr1", 128,
                lambda blk: xh3[:, blk * BLKC:(blk + 1) * BLKC],
                big_a, c_big)

            # --- phase B: encoder L2 (h1_seq -> z in h2a, in place) ---
            nc.vector.memset(h2a[H2:H2 + 1, :], 1.0)
            nc.vector.memset(h2a[0:H2, :], 0.0)
            nc.vector.memset(c_sm[:], 0.0)
            wr2_g = lambda g: wslice("wr2", 65, g, H2)
            for blk in range(NBLK):
                ps = psum_pool.tile([H2, 4, BLKC], f32, tag="ps")
                xr = big_a[:, blk * BLKC + BC:(blk + 1) * BLKC + BC]
                for g in range(4):
                    nc.tensor.matmul(
                        ps[:, g, :], wslice("wk2", 128, g, H2), xr,
                        start=True, stop=False, skip_group_check=True,
                    )
                for s in range(SB):
                    cs = slice(s * BC, (s + 1) * BC)
                    lstm_step(nc, ps, cs, H2, wr2_g, h2a[:], c_sm,
                              h2a[0:H2, :])

            # --- phase C: decoder L1 (z -> h3_seq in xh3), const input ---
            nc.vector.memset(xh3[H2:H2 + 1, :], 1.0)
            for s in range(SB):
                nc.vector.tensor_copy(z_rep[:, s * BC:(s + 1) * BC], h2a[:])

            lstm_phase(
                H2, "wd1k", 65, "wd1r", 64,
                lambda blk: z_rep[:],
                xh3, c_sm)

            # --- phase D: decoder L2 (h3_seq -> h4_seq in big_a) ---
            lstm_phase(
                H1, "wd2k", 65, "wd2r", 128,
                lambda blk: xh3[:, blk * BLKC + BC:(blk + 1) * BLKC + BC],
                big_a, c_big)

            # --- phase E: TimeDistributed Dense (h4_seq -> out) ---
            w_out = wp[0:128, WOFF["wout"]:WOFF["wout"] + F]
            b_out = wp[0:F, WOFF["bout"]:WOFF["bout"] + 1]
            for blk in range(NBLK):
                pd = psum_pool.tile([F, BLKC], f32, tag="ps")
                nc.tensor.matmul(
                    pd[:], w_out,
                    big_a[:, blk * BLKC + BC:(blk + 1) * BLKC + BC],
                    start=True, stop=True,
                )
                ob = work.tile([F, BLKC], f32, tag="ob")
                nc.scalar.activation(ob[:], pd[:], AF.Identity, bias=b_out)
                nc.sync.dma_start(out_d[:, blk * BLKC:(blk + 1) * BLKC], ob[:])

    _split_excess_waits(nc, mybir)
    return nc


def _split_excess_waits(nc, mybir, limits=None):
    """walrus's PE codegen (S3_LW struct) accepts a single sync-wait per
    matmul; Tile sometimes emits 2+. Move excess waits onto a preceding
    sequencer NoOp on the same engine (executed in order before the
    instruction, so semantics are preserved)."""
    exempt = ()
    for bb in nc.main_func.blocks:
        il = bb.instructions
        pos = 0
        while pos < len(il):
            ins = il[pos]
            limit = None if isinstance(ins, exempt) else 1
            si = ins.sync_info
            if limit is not None and si is not None and len(si.on_wait) > limit:
                keep = list(si.on_wait)[-limit:]
                spill = list(si.on_wait)[:-limit]
                for w in spill:
                    nop = mybir.InstNoOp(
                        name=nc.get_next_instruction_name(),
                        text_hint="wait_split",
                        engine=ins.engine,
                        bass_nofuse=True,
                        sync_info=mybir.SyncInfo(on_wait=[w], on_update=[]),
                    )
                    il.insert(pos, nop)
                    pos += 1
                ins.sync_info = mybir.SyncInfo(
                    on_wait=keep, on_update=list(si.on_update))
            pos += 1


def _get_nc():
    if "nc" not in _CACHE:
        _CACHE["nc"] = _build()
    return _CACHE["nc"]


def _prep_weights(Wk1, Wr1, b1, Wk2, Wr2, b2, Wd1k, Wd1r, bd1, Wd2k, Wd2r,
                  bd2, Wout, bout):
    def perm(W, H):
        Din = W.shape[0]
        return W.reshape(Din, 4, H)[:, PERM, :].reshape(Din, 4 * H)

    def aug(W, b, H):
        return perm(np.concatenate([W, b[None, :]], axis=0), H)

    mats = {
        "wk1": aug(Wk1, b1, H1),
        "wr1": perm(Wr1, H1),
        "wk2": perm(Wk2, H2),
        "wr2": aug(Wr2, b2, H2),
        "wd1k": aug(Wd1k, bd1, H2),
        "wd1r": perm(Wd1r, H2),
        "wd2k": aug(Wd2k, bd2, H1),
        "wd2r": perm(Wd2r, H1),
        "wout": Wout,
        "bout": np.asarray(bout).reshape(F, 1),
    }
    wpack = np.zeros((128, WCOLS), np.float32)
    for name, rows, cols in WSEGS:
        m = np.asarray(mats[name], np.float32)
        assert m.shape == (rows, cols), (name, m.shape)
        wpack[0:rows, WOFF[name]:WOFF[name] + cols] = m
    return wpack


def kernel(x, Wk1, Wr1, b1, Wk2, Wr2, b2, Wd1k, Wd1r, bd1, Wd2k, Wd2r, bd2,
           Wout, bout, _run_kwargs=None):
    from concourse.bass_utils import run_bass_kernel_spmd

    nc = _get_nc()
    wpack = _prep_weights(
        np.asarray(Wk1), np.asarray(Wr1), np.asarray(b1),
        np.asarray(Wk2), np.asarray(Wr2), np.asarray(b2),
        np.asarray(Wd1k), np.asarray(Wd1r), np.asarray(bd1),
        np.asarray(Wd2k), np.asarray(Wd2r), np.asarray(bd2),
        np.asarray(Wout), np.asarray(bout))

    x = np.asarray(x, dtype=np.float32)
    in_maps = []
    for i in range(NCORES):
        xs = x[i * BC:(i + 1) * BC]                 # [32, 512, 64]
        xt = xs.transpose(2, 1, 0).reshape(F, NT)   # [64, (t,b)]
        xaug = np.concatenate([xt, np.ones((1, NT), np.float32)], axis=0)
        in_maps.append({"xa": np.ascontiguousarray(xaug), "wpack": wpack})

    kwargs = _run_kwargs or {}
    res = run_bass_kernel_spmd(nc, in_maps, list(range(NCORES)), **kwargs)
    _CACHE["last_results"] = res

    out = np.empty((B, T, F), np.float32)
    for i in range(NCORES):
        o = np.asarray(res.results[i]["out"]).reshape(F, T, BC)
        out[i * BC:(i + 1) * BC] = o.transpose(2, 1, 0)
    return out



# revision 9
# speedup vs baseline: 3.1788x; 3.1788x over previous
"""LSTM autoencoder (4-layer + TimeDistributed Dense) on 8 TRN2 NeuronCores.

Sharding: data-parallel over batch (B=256 -> 32 samples/core), weights
replicated. Per-core layout keeps everything "transposed": states are
[H partitions, batch free], so the recurrent matmul is
  z^T[gate] = W[:, gate]^T @ h^T   (weights stationary, state moving, N=32)
and the gate nonlinearities/cell updates run on [H, 32] tiles.

All matmul operands are bf16 (fp32 would lower to 2 half-speed LDW+MM
pairs each); PSUM accumulation stays fp32, the cell state c stays fp32.
h is written as bf16 directly by the final vector op of each step.

Per 8-step block, the input-side (Wk) matmuls for all 8 steps are batched
into one N=256 matmul per gate plane, exploiting has_written accumulation:
the recurrent matmuls then accumulate per-step N=32 slices on top.

The two encoder layers are interleaved at step granularity with a
one-block skew (L2 consumes block b-1 of h1 while L1 produces block b),
so L2's matmuls fill the PE while L1's activations/vector ops run and
vice versa; same for the two decoder layers. The TimeDistributed Dense
runs as a pipelined tail phase.

Gate plane order in PSUM is (g, i, f, o) — one sigmoid activation covers
planes 1:4; relu(g) is fused into the scalar_tensor_tensor
u = max(g, 0) * i on the vector engine. relu(c) == c identically because
c >= 0 by induction (g >= 0 post-relu, i,f in (0,1), c0 = 0), so h = o*c.

Biases are folded in via a ones-row augmentation of the moving operand
(K -> K+1) on whichever side has K < 128.
"""

import numpy as np

B, T, F, H1, H2 = 256, 512, 64, 128, 64
NCORES = 8
BC = B // NCORES          # 32 samples per core
NT = T * BC               # 16384 columns in time-major (t, b) layout
SB = 8                    # recurrence steps per PSUM block
NBLK = T // SB            # 64 blocks
BLKC = SB * BC            # 256 columns per block
PERM = [2, 0, 1, 3]       # keras (i,f,g,o) -> planes (g,i,f,o)

# wpack column offsets: (name, rows, cols)
WSEGS = [("wk1", 65, 512), ("wr1", 128, 512), ("wk2", 128, 256),
         ("wr2", 65, 256), ("wd1k", 65, 256), ("wd1r", 64, 256),
         ("wd2k", 65, 512), ("wd2r", 128, 512), ("wout", 128, 64)]
WOFF = {}
_o = 0
for _n, _p, _c in WSEGS:
    WOFF[_n] = _o
    _o += _c
WCOLS = _o

_CACHE = {}


def _build():
    import concourse.bass as bass
    import concourse.mybir as mybir
    import concourse.tile as tile

    f32 = mybir.dt.float32
    bf16 = mybir.dt.bfloat16
    AF = mybir.ActivationFunctionType
    ALU = mybir.AluOpType

    nc = bass.Bass()

    xa = nc.dram_tensor("xa", [F + 1, NT], bf16, kind="ExternalInput")
    wp_d = nc.dram_tensor("wpack", [128, WCOLS], bf16, kind="ExternalInput")
    bo_d = nc.dram_tensor("bo", [F, 1], f32, kind="ExternalInput")
    out_d = nc.dram_tensor("out", [F, NT], f32, kind="ExternalOutput")

    with tile.TileContext(nc) as tc:
        with (
            tc.tile_pool(name="singles", bufs=1) as singles,
            tc.tile_pool(name="work", bufs=4) as work,
            tc.tile_pool(name="psA", bufs=2, space="PSUM") as psA,
            tc.tile_pool(name="psB", bufs=2, space="PSUM") as psB,
        ):
            wp = singles.tile([128, WCOLS], bf16, tag="wp")
            nc.sync.dma_start(wp[:], wp_d[:])
            b_out = singles.tile([F, 1], f32, tag="bo")
            nc.sync.dma_start(b_out[:], bo_d[:])

            def wslice(name, rows, g, H):
                o = WOFF[name]
                return wp[0:rows, o + g * H: o + (g + 1) * H]

            # --- state buffers (all bf16 except cell states) ---
            # big_a serves as h1_seq (enc) then h4_seq (dec).
            # Column layout: col (t+1)*32 .. +32 holds h_t; cols 0:32 zero.
            # xh3 serves as the full x input (enc, cols t*32 directly)
            # then as h3_seq (dec, cols shifted by +BC).
            big_a = singles.tile([H1, NT + BC], bf16, tag="big_a")
            xh3 = singles.tile([H2 + 1, NT + BC], bf16, tag="xh3")
            h2a = singles.tile([H2 + 1, BC], bf16, tag="h2a")
            z_rep = singles.tile([H2 + 1, BLKC], bf16, tag="z_rep")
            c_big = singles.tile([H1, BC], f32, tag="c_big")
            c_sm = singles.tile([H2, BC], f32, tag="c_sm")

            nc.sync.dma_start(xh3[:, 0:NT], xa[:])

            def lstm_step(ps, cs, H, wr_g, hprev, c_t, h_out, atag, utag):
                """One recurrence step given psum block ps / col slice cs."""
                for g in range(4):
                    nc.tensor.matmul(
                        ps[:, g, cs], wr_g(g), hprev,
                        start=False, stop=True, skip_group_check=True,
                    )
                act = work.tile([H, 3, BC], f32, tag=atag)
                nc.scalar.activation(act[:], ps[:, 1:4, cs], AF.Sigmoid)
                u = work.tile([H, BC], f32, tag=utag)
                # u = relu(g) * i  (relu fused into the STT, off the ACT queue)
                nc.vector.scalar_tensor_tensor(
                    u[:], ps[:, 0, cs], 0.0, act[:, 0, :], ALU.max, ALU.mult)
                nc.vector.tensor_mul(c_t[:], act[:, 1, :], c_t[:])
                nc.vector.tensor_add(c_t[:], c_t[:], u[:])
                nc.vector.tensor_mul(h_out, act[:, 2, :], c_t[:])

            def enc_dec_phase(wkA, wkA_rows, wrA, wrA_rows,
                              xA_of_blk, hseqA, cA,
                              wkB, wkB_rows, wrB, wrB_rows,
                              xB_of_blk, hB_of, hB_out, cB):
                """Two stacked LSTM layers, step-interleaved, one-block skew.

                Layer A (H1-wide) feeds layer B (H2-wide). xB_of_blk(blk)
                gives B's batched input operand (block blk of A's output).
                hB_of(t)/hB_out(t) give B's recurrent operand / h target.
                """
                wrA_g = lambda g: wslice(wrA, wrA_rows, g, H1)
                wrB_g = lambda g: wslice(wrB, wrB_rows, g, H2)
                for blk in range(NBLK + 1):
                    if blk < NBLK:
                        pa = psA.tile([H1, 4, BLKC], f32, tag="psA")
                        xr = xA_of_blk(blk)
                        for g in range(4):
                            # start only on the first plane of each 2KB PSUM
                            # bank: start=True clears has_written for the
                            # WHOLE bank, and two 1KB gate planes share one.
                            nc.tensor.matmul(
                                pa[:, g, :], wslice(wkA, wkA_rows, g, H1), xr,
                                start=(g % 2 == 0), stop=False,
                                skip_group_check=True,
                            )
                    if blk >= 1:
                        pb = psB.tile([H2, 4, BLKC], f32, tag="psB")
                        xrb = xB_of_blk(blk - 1)
                        for g in range(4):
                            nc.tensor.matmul(
                                pb[:, g, :], wslice(wkB, wkB_rows, g, H2), xrb,
                                start=(g % 2 == 0), stop=False,
                                skip_group_check=True,
                            )
                    for s in range(SB):
                        if blk < NBLK:
                            t = blk * SB + s
                            lstm_step(
                                pa, slice(s * BC, (s + 1) * BC), H1, wrA_g,
                                hseqA[:, t * BC:(t + 1) * BC], cA,
                                hseqA[:, (t + 1) * BC:(t + 2) * BC],
                                "actA", "uA")
                        if blk >= 1:
                            t2 = (blk - 1) * SB + s
                            lstm_step(
                                pb, slice(s * BC, (s + 1) * BC), H2, wrB_g,
                                hB_of(t2), cB, hB_out(t2), "actB", "uB")

            # ================= encoder: L1 + L2 interleaved =================
            nc.vector.memset(c_big[:], 0.0)
            nc.vector.memset(c_sm[:], 0.0)
            nc.vector.memset(big_a[:, 0:BC], 0.0)
            nc.vector.memset(h2a[H2:H2 + 1, :], 1.0)
            nc.vector.memset(h2a[0:H2, :], 0.0)

            enc_dec_phase(
                "wk1", 65, "wr1", 128,
                lambda blk: xh3[:, blk * BLKC:(blk + 1) * BLKC],
                big_a, c_big,
                "wk2", 128, "wr2", 65,
                lambda blk: big_a[:, blk * BLKC + BC:(blk + 1) * BLKC + BC],
                lambda t2: h2a[:],
                lambda t2: h2a[0:H2, :],
                c_sm)

            # ============== decoder prep: z_rep, h3 seq init ===============
            for s in range(SB):
                nc.vector.tensor_copy(z_rep[:, s * BC:(s + 1) * BC], h2a[:])
            nc.vector.memset(xh3[0:H2, 0:BC], 0.0)
            nc.vector.memset(xh3[H2:H2 + 1, NT:NT + BC], 1.0)
            nc.vector.memset(c_big[:], 0.0)
            nc.vector.memset(c_sm[:], 0.0)
            nc.vector.memset(big_a[:, 0:BC], 0.0)

            # ================= decoder: D1 + D2 interleaved =================
            # D1 (H2-wide, producer of h3) runs on block blk; D2 (H1-wide,
            # consumer) lags one block behind on blk-1.
            wrA_g = lambda g: wslice("wd2r", 128, g, H1)
            wrB_g = lambda g: wslice("wd1r", 64, g, H2)
            for blk in range(NBLK + 1):
                if blk < NBLK:
                    pb = psB.tile([H2, 4, BLKC], f32, tag="psB")
                    for g in range(4):
                        nc.tensor.matmul(
                            pb[:, g, :], wslice("wd1k", 65, g, H2), z_rep[:],
                            start=(g % 2 == 0), stop=False,
                            skip_group_check=True,
                        )
                if blk >= 1:
                    pa = psA.tile([H1, 4, BLKC], f32, tag="psA")
                    xr = xh3[:, (blk - 1) * BLKC + BC:blk * BLKC + BC]
                    for g in range(4):
                        nc.tensor.matmul(
                            pa[:, g, :], wslice("wd2k", 65, g, H1), xr,
                            start=(g % 2 == 0), stop=False,
                            skip_group_check=True,
                        )
                for s in range(SB):
                    if blk < NBLK:
                        t = blk * SB + s
                        lstm_step(
                            pb, slice(s * BC, (s + 1) * BC), H2, wrB_g,
                            xh3[0:H2, t * BC:(t + 1) * BC], c_sm,
                            xh3[0:H2, (t + 1) * BC:(t + 2) * BC],
                            "actB", "uB")
                    if blk >= 1:
                        t2 = (blk - 1) * SB + s
                        lstm_step(
                            pa, slice(s * BC, (s + 1) * BC), H1, wrA_g,
                            big_a[:, t2 * BC:(t2 + 1) * BC], c_big,
                            big_a[:, (t2 + 1) * BC:(t2 + 2) * BC],
                            "actA", "uA")

            # ============ dense tail: h4_seq @ Wout + bout -> out ===========
            w_out = wp[0:128, WOFF["wout"]:WOFF["wout"] + F]
            for blk in range(NBLK):
                pd = psA.tile([H1, 4, BLKC], f32, tag="psA")
                nc.tensor.matmul(
                    pd[0:F, 0, :], w_out,
                    big_a[:, blk * BLKC + BC:(blk + 1) * BLKC + BC],
                    start=True, stop=True,
                )
                ob = work.tile([F, BLKC], f32, tag="ob")
                nc.scalar.activation(ob[:], pd[0:F, 0, :], AF.Identity,
                                     bias=b_out[:])
                nc.sync.dma_start(out_d[:, blk * BLKC:(blk + 1) * BLKC], ob[:])

    _split_excess_waits(nc, mybir)
    return nc


def _split_excess_waits(nc, mybir, limits=None):
    """walrus's PE codegen (S3_LW struct) accepts a single sync-wait per
    matmul; Tile sometimes emits 2+. Move excess waits onto a preceding
    sequencer NoOp on the same engine (executed in order before the
    instruction, so semantics are preserved)."""
    exempt = ()
    for bb in nc.main_func.blocks:
        il = bb.instructions
        pos = 0
        while pos < len(il):
            ins = il[pos]
            limit = None if isinstance(ins, exempt) else 1
            si = ins.sync_info
            if limit is not None and si is not None and len(si.on_wait) > limit:
                keep = list(si.on_wait)[-limit:]
                spill = list(si.on_wait)[:-limit]
                for w in spill:
                    nop = mybir.InstNoOp(
                        name=nc.get_next_instruction_name(),
                        text_hint="wait_split",
                        engine=ins.engine,
                        bass_nofuse=True,
                        sync_info=mybir.SyncInfo(on_wait=[w], on_update=[]),
                    )
                    il.insert(pos, nop)
                    pos += 1
                ins.sync_info = mybir.SyncInfo(
                    on_wait=keep, on_update=list(si.on_update))
            pos += 1


def _get_nc():
    if "nc" not in _CACHE:
        _CACHE["nc"] = _build()
    return _CACHE["nc"]


def _prep_weights(Wk1, Wr1, b1, Wk2, Wr2, b2, Wd1k, Wd1r, bd1, Wd2k, Wd2r,
                  bd2, Wout, bout):
    import ml_dtypes

    def perm(W, H):
        Din = W.shape[0]
        return W.reshape(Din, 4, H)[:, PERM, :].reshape(Din, 4 * H)

    def aug(W, b, H):
        return perm(np.concatenate([W, b[None, :]], axis=0), H)

    mats = {
        "wk1": aug(Wk1, b1, H1),
        "wr1": perm(Wr1, H1),
        "wk2": perm(Wk2, H2),
        "wr2": aug(Wr2, b2, H2),
        "wd1k": aug(Wd1k, bd1, H2),
        "wd1r": perm(Wd1r, H2),
        "wd2k": aug(Wd2k, bd2, H1),
        "wd2r": perm(Wd2r, H1),
        "wout": Wout,
    }
    wpack = np.zeros((128, WCOLS), np.float32)
    for name, rows, cols in WSEGS:
        m = np.asarray(mats[name], np.float32)
        assert m.shape == (rows, cols), (name, m.shape)
        wpack[0:rows, WOFF[name]:WOFF[name] + cols] = m
    return wpack.astype(ml_dtypes.bfloat16)


def kernel(x, Wk1, Wr1, b1, Wk2, Wr2, b2, Wd1k, Wd1r, bd1, Wd2k, Wd2r, bd2,
           Wout, bout, _run_kwargs=None):
    import ml_dtypes
    from concourse.bass_utils import run_bass_kernel_spmd

    nc = _get_nc()
    wpack = _prep_weights(
        np.asarray(Wk1), np.asarray(Wr1), np.asarray(b1),
        np.asarray(Wk2), np.asarray(Wr2), np.asarray(b2),
        np.asarray(Wd1k), np.asarray(Wd1r), np.asarray(bd1),
        np.asarray(Wd2k), np.asarray(Wd2r), np.asarray(bd2),
        np.asarray(Wout), np.asarray(bout))
    bo = np.asarray(bout, np.float32).reshape(F, 1)

    x = np.asarray(x, dtype=np.float32)
    in_maps = []
    for i in range(NCORES):
        xs = x[i * BC:(i + 1) * BC]                 # [32, 512, 64]
        xt = xs.transpose(2, 1, 0).reshape(F, NT)   # [64, (t,b)]
        xaug = np.concatenate([xt, np.ones((1, NT), np.float32)], axis=0)
        xaug = np.ascontiguousarray(xaug).astype(ml_dtypes.bfloat16)
        in_maps.append({"xa": xaug, "wpack": wpack, "bo": bo})

    kwargs = _run_kwargs or {}
    res = run_bass_kernel_spmd(nc, in_maps, list(range(NCORES)), **kwargs)
    _CACHE["last_results"] = res

    out = np.empty((B, T, F), np.float32)
    for i in range(NCORES):
        o = np.asarray(res.results[i]["out"]).reshape(F, T, BC)
        out[i * BC:(i + 1) * BC] = o.transpose(2, 1, 0)
    return out
